# revision 3
# baseline (speedup 1.0000x reference)
"""FP8 batch-matmul-dense kernel for Trainium2 (8 NeuronCores, batch-sharded).

Problem: out[b] = fp8qdq(x)[b] @ fp8qdq(w)[b] + bias[b]
  x: [32, 512, 2048] f32, w: [32, 2048, 2048] f32, bias: [32, 1, 2048] f32
  fp8qdq = torchao-style dynamic tensorwise scaling: s = 448/amax(|t|),
  q = e4m3fn(t*s), dq = q/s. Global (whole-tensor) amax.

Sharding: batch axis across 8 cores, 4 slices each (expert-parallel style).

Single fused NEFF. Phase A streams x then w at fp32 computing exact local
amaxes on DVE; amax_x and amax_w are AllReduce(max)'d across the cores (a
dummy warmup AllReduce pays the first-collective setup under the x loads).
x is PE-transposed and quantized into 4MiB of resident fp8 lhsT codes.
Phase B re-reads w, quantizes on DVE, runs DoubleRow fp8 matmuls (fp32 PSUM
accum) in mt-pair sweeps over 8 PSUM banks, drains bias+rescale to bf16 and
stores via SWDGE (host upcasts; bf16's 2^-9 rounding is invisible at the
2e-2 gate).

Performance model (from extensive ntff profiling of prior revisions):
  - The binding resource is the 16 SDMA engines (~22GB/s each on 8-16KB
    descriptors): the kernel must keep them 100% fed. All w moves as
    [128, 2, N] "row-pair" tiles (partition p = DRAM rows 2p/2p+1, one
    16KB-contiguous descriptor per partition) through a 4-buffer stage pool
    (deep enough that the DVE amax reduce that recycles a slot never stalls
    the queue). The matmul consumes the pair layout directly: k-group t
    pairs k = 256t + 2*ki + par, and the x-transposes read stride-2 column
    slices so the lhsT pairing matches.
  - SBUF is exactly full: to afford 4 stage buffers, only x batches 0..2
    are held resident ([128,4,2048] tiles); batch 3's x streams through the
    stage pool for amax and is re-read during the ARw collective window
    (where the DMA would otherwise idle) as two [128,2,2048] row-pair tiles
    whose pair-packed m-order is fixed up in the output store rearrange.
  - Engine queues are strict FIFO: sx ops sit ~14 staged reduces deep (a
    collective against a busy SDMA path takes ~50us); phase-B quants run
    only on DVE so the ACT queue (which issues half the DMA triggers) never
    head-of-line blocks on sw; 6 re-read loads are emitted before the first
    quant as a prefetch prologue across the ARw window; bias broadcasts are
    emitted after the ARw chain.

Quantization math (exact match to the reference): s' = 224/amax
  (= fl(448/amax)/2 exactly) because TRN fp8_e4m3 tops out at 240, not 448:
  the OCP e4m3fn lattice scaled by 1/2 lands exactly on the TRN lattice.
  Matmul runs on raw fp8 codes (exact products, fp32 PSUM accum); output is
  rescaled by c = 1/(sx'*sw'). Scales come from nc.vector.reciprocal
  on-device; 1-2 ulp deviation vs host fp32 divide perturbs ~1e-6 of the
  fp8 codes by 1 ulp - invisible at the gate.

Per-core HBM traffic: 16 (x) + 4 (x b3 re-read, hidden in the ARw window)
+ 64 (w) + 64 (w re-read) + 8 (out bf16) = 156MiB, one NEFF ramp.
"""

import os
import sys

for _p in ("/root/.axon_site", "/root/.axon_site/_ro/trn_rl_repo", "/opt/trn_rl_repo"):
    if os.path.isdir(_p) and _p not in sys.path:
        sys.path.append(_p)

import numpy as np

import concourse.bass as bass
import concourse.bass_isa as bass_isa
import concourse.mybir as mybir
import concourse.tile as tile
from concourse import bacc
from concourse.bass_utils import run_bass_kernel_spmd
from concourse.masks import make_identity

# Problem shape (hardcoded per contest rules).
B, M, K, N = 32, 512, 2048, 2048
NCORES = 8
BL = B // NCORES          # 4 batch slices per core
P = 128
KT = K // P               # 16 k-tiles per batch
KP = KT // 2              # 8 k-groups (256 rows, row-pair packed) per batch
MT = M // P               # 4 m-tiles
NFREE = 512               # matmul moving free dim (one PSUM bank)
NT = N // NFREE           # 4 n-tiles
XRES = 3                  # x batches held resident; batch 3 streams
SX_DEPTH = 16             # staged (2MiB) reduces before sx in the DVE FIFO
PREFETCH = 6              # phase-B loads emitted before the first quant
FP8_HALF_MAX = 224.0      # 448/2: OCP grid mapped onto TRN e4m3

F32 = mybir.dt.float32
BF16 = mybir.dt.bfloat16
FP8 = mybir.dt.float8e4

_cache = {}


def _build_fused_nc(with_bias=True):
    nc = bacc.Bacc("TRN2", target_bir_lowering=False, debug=False, num_devices=NCORES)
    x = nc.dram_tensor("x", [BL, M, K], F32, kind="ExternalInput")
    w = nc.dram_tensor("w", [BL, K, N], F32, kind="ExternalInput")
    bias = nc.dram_tensor("bias", [BL, 1, N], F32, kind="ExternalInput")
    consts = nc.dram_tensor("consts", [1, 2], F32, kind="ExternalInput")
    out = nc.dram_tensor("out", [BL, M, N], BF16, kind="ExternalOutput")

    rg = [list(range(NCORES))]
    nld = [0]   # load counter for HWDGE ring alternation

    def ring():
        # All load triggers ride the sync HWDGE ring so the scalar (ACT)
        # queue stays free for the xqt copies: a sx-gated copy behind a
        # load trigger would otherwise pin the transposes (and the xbig
        # release) to the end of the w stream.
        nld[0] += 1
        return nc.sync

    def w_pair_src(b, t):
        """w[b] rows [256t, 256t+256) as [128, 2, N]: partition p holds DRAM
        rows 2p/2p+1 -> one 16KB-contiguous descriptor per partition."""
        return w[b, t * 2 * P:(t + 1) * 2 * P, :].rearrange(
            "(p r) n -> p r n", r=2
        )

    def x_pair_src(s):
        """x[3] rows [256s, 256s+256) as [128, 2, K] row-pair tiles."""
        return x[BL - 1, s * 2 * P:(s + 1) * 2 * P, :].rearrange(
            "(p r) n -> p r n", r=2
        )

    with tile.TileContext(nc) as tc:
        with (
            tc.tile_pool(name="small", bufs=1) as small,
            tc.tile_pool(name="acc", bufs=1) as accp,
            tc.tile_pool(name="xqt", bufs=1) as xqtp,
            tc.tile_pool(name="wstage", bufs=4) as wstage,
            tc.tile_pool(name="dram", bufs=6, space="DRAM") as dram,
        ):
            ident = small.tile([P, P], F32, name="ident")
            make_identity(nc, ident[:])
            cst = small.tile([1, 2], F32, name="cst")
            nc.sync.dma_start(cst[:], consts[0:1, :])
            # scl slots: 0=1/ax, 1=sx, 2=1/aw, 3=sw, 4=sx*sw, 5=c
            scl = small.tile([1, 8], F32, name="scl")
            axg = small.tile([1, 1], F32, name="axg")
            awg = small.tile([1, 1], F32, name="awg")
            cb = small.tile([P, 4], F32, name="cb")   # 0=sx, 1=sw, 2=c

            acc = accp.tile([P, 8 + BL * KP], F32, name="acc")
            red = accp.tile([P, 2], F32, name="red")
            par = accp.tile([P, 2], F32, name="par")

            # resident fp8 lhsT codes: [ki, t, par, b*M + u*128 + c] where
            # (ki, par) pair k = 256t + 2*ki + par (matching the w pairing)
            # and unit u is the m-block (plain for b0..2, (s,r)-pair for b3).
            xqt = xqtp.tile([P, KP, 2, BL * M], FP8, name="xqt")

            dum_in = dram.tile([1, 8], F32, name="dum_in")
            dum_out = dram.tile([1, 8], F32, name="dum_out")
            arx_in = dram.tile([1, 8], F32, name="arx_in")
            arx_out = dram.tile([1, 8], F32, name="arx_out")
            arw_in = dram.tile([1, 8], F32, name="arw_in")
            arw_out = dram.tile([1, 8], F32, name="arw_out")

            # warmup collective: pays the ~50us first-collective setup while
            # the x/w loads stream.
            nc.gpsimd.dma_start(dum_in[0:1, 0:2], cst[:])
            nc.gpsimd.collective_compute(
                "AllReduce", mybir.AluOpType.max, replica_groups=rg,
                ins=[dum_in.opt()], outs=[dum_out.opt()],
            )

            xbig = tc.alloc_tile_pool(name="xbig", bufs=XRES)
            trps = tc.alloc_tile_pool(name="trps", bufs=3, space="PSUM")

            # ---- x batch 3 first: streams through the stage pool for amax
            # only. Going first keeps its DVE reduces clear of the staged-w
            # reduce chain that recycles the stage slots. ----
            for s in range(2):
                st = wstage.tile([P, 2, K], F32, name="ws", tag="ws")
                ring().dma_start(st[:], x_pair_src(s))
                nc.vector.tensor_reduce(
                    acc[:, XRES + s:XRES + s + 1], st[:],
                    axis=mybir.AxisListType.XY, op=mybir.AluOpType.max,
                    apply_absolute_value=True,
                )
            # ---- x batches 0..2: resident loads + amax (one reduce per
            # tile: the ~1.2us/instruction DVE overhead dominates otherwise) --
            xs_tiles = []
            for b in range(XRES):
                t = xbig.tile([P, 4, K], F32, name="xs", tag="xs")
                src = x[b, :, :].rearrange("(p k) n -> k p n", p=4)
                ring().dma_start(t[:], src)
                nc.vector.tensor_reduce(
                    acc[:, b:b + 1], t[:],
                    axis=mybir.AxisListType.XY, op=mybir.AluOpType.max,
                    apply_absolute_value=True,
                )
                xs_tiles.append(t)

            # ---- amax_x AllReduce trigger (result consumed later) ----
            nc.vector.tensor_reduce(
                red[:, 0:1], acc[:, 0:XRES + 2],
                axis=mybir.AxisListType.X, op=mybir.AluOpType.max,
            )
            nc.gpsimd.partition_all_reduce(
                par[:, 0:1], red[:, 0:1], channels=P,
                reduce_op=bass_isa.ReduceOp.max,
            )
            nc.gpsimd.dma_start(arx_in[0:1, 0:1], par[0:1, 0:1])
            nc.gpsimd.collective_compute(
                "AllReduce", mybir.AluOpType.max, replica_groups=rg,
                ins=[arx_in.opt()], outs=[arx_out.opt()],
            )
            nc.gpsimd.dma_start(axg[:], arx_out[0:1, 0:1])

            col = [8]

            def stage_w_load(b, t):
                ws = wstage.tile([P, 2, N], F32, name="ws", tag="ws")
                ring().dma_start(ws[:], w_pair_src(b, t))
                nc.vector.tensor_reduce(
                    acc[:, col[0]:col[0] + 1], ws[:],
                    axis=mybir.AxisListType.XY, op=mybir.AluOpType.max,
                    apply_absolute_value=True,
                )
                col[0] += 1

            staged_plan = [(b, t) for b in range(BL) for t in range(KP)]
            for b_, t_ in staged_plan[:SX_DEPTH]:
                stage_w_load(b_, t_)

            # sx = 224 / max(amax_x, 1e-12): DVE reaches this ~14 staged
            # reduces deep, by when the AllReduce result has landed.
            nc.vector.tensor_scalar_max(axg[:], axg[:], 1e-12)
            nc.vector.reciprocal(scl[0:1, 0:1], axg[:])
            nc.vector.tensor_scalar_mul(scl[0:1, 1:2], scl[0:1, 0:1], FP8_HALF_MAX)
            nc.gpsimd.partition_broadcast(cb[:, 0:1], scl[0:1, 1:2])
            sx_ap = cb[:, 0:1]

            for b_, t_ in staged_plan[SX_DEPTH:]:
                stage_w_load(b_, t_)

            # ---- x batch 3 re-read (runs inside the ARw collective window,
            # where the stream would otherwise idle) ----
            xb3_tiles = []
            for s in range(2):
                st = wstage.tile([P, 2, K], F32, name="ws", tag="ws")
                ring().dma_start(st[:], x_pair_src(s))
                xb3_tiles.append(st)

            # ---- x transposes, pair-strided to match the w pairing:
            # psum partition ki of group (t,par) holds k = 256t + 2ki + par.
            def xpose_group(b, t, parp, srcs):
                # srcs: 4 (view, unit) pairs -> one [P, 512] psum -> xqt
                ps = trps.tile([P, M], F32, name="tps", tag="tps")
                for v, u in srcs:
                    nc.tensor.transpose(
                        ps[:, u * P:(u + 1) * P],
                        v[:, parp, t * P:(t + 1) * P],
                        ident[:],
                    )
                nc.scalar.activation(
                    xqt[:, t, parp, b * M:(b + 1) * M], ps[:],
                    mybir.ActivationFunctionType.Copy, scale=sx_ap,
                )

            for b in range(XRES):
                views = [
                    xs_tiles[b][:, j, :].rearrange("p (k two) -> p two k", two=2)
                    for j in range(MT)
                ]
                for t in range(KP):
                    for parp in range(2):
                        xpose_group(b, t, parp, [(views[j], j) for j in range(MT)])
            xbig.release()

            # phase-B pools go into the released xbig zone; the no-bias
            # variant spends the freed bias budget on a deeper re-read
            # prefetch across the ARw collective window.
            restage = tc.alloc_tile_pool(
                name="restage", bufs=2 if with_bias else 3
            )
            wqp = tc.alloc_tile_pool(name="wq", bufs=12 if with_bias else 11)
            ostp = tc.alloc_tile_pool(name="ost", bufs=2)
            if with_bias:
                bias1p = tc.alloc_tile_pool(name="bias1", bufs=1)
                biasbp = tc.alloc_tile_pool(name="biasb", bufs=2)
            prefetch = PREFETCH if with_bias else PREFETCH + 1

            # ---- amax_w AllReduce ----
            nc.vector.tensor_reduce(
                red[:, 1:2], acc[:, 8:col[0]],
                axis=mybir.AxisListType.X, op=mybir.AluOpType.max,
            )
            nc.gpsimd.partition_all_reduce(
                par[:, 1:2], red[:, 1:2], channels=P,
                reduce_op=bass_isa.ReduceOp.max,
            )
            nc.gpsimd.dma_start(arw_in[0:1, 0:1], par[0:1, 1:2])
            nc.gpsimd.collective_compute(
                "AllReduce", mybir.AluOpType.max, replica_groups=rg,
                ins=[arw_in.opt()], outs=[arw_out.opt()],
            )
            nc.gpsimd.dma_start(awg[:], arw_out[0:1, 0:1])
            # sw = 224 / max(amax_w, 1e-12); c = 1/(sx*sw)
            nc.vector.tensor_scalar_max(awg[:], awg[:], 1e-12)
            nc.vector.reciprocal(scl[0:1, 2:3], awg[:])
            nc.vector.tensor_scalar_mul(scl[0:1, 3:4], scl[0:1, 2:3], FP8_HALF_MAX)
            nc.vector.tensor_tensor(
                scl[0:1, 4:5], scl[0:1, 1:2], scl[0:1, 3:4],
                mybir.AluOpType.mult,
            )
            nc.vector.reciprocal(scl[0:1, 5:6], scl[0:1, 4:5])
            nc.gpsimd.partition_broadcast(cb[:, 1:2], scl[0:1, 3:4])
            nc.gpsimd.partition_broadcast(cb[:, 2:3], scl[0:1, 5:6])
            sw_ap = cb[:, 1:2]
            c_ap = cb[:, 2:3]

            # ---- phase B: software-pipelined re-read + quantize + mm ----
            flat = [(b, t) for b in range(BL) for t in range(KP)]
            stage_tiles = {}

            def issue_load(i):
                # phase-B loads ride the sync ring only: the ACT queue hosts
                # half the w-quants, and a sw-gated quant ahead of a DMA
                # trigger would head-of-line block it.
                b_, t_ = flat[i]
                pool = restage if i % 3 == 2 else wstage
                st = pool.tile([P, 2, N], F32, name="ws", tag="ws")
                ring().dma_start(st[:], w_pair_src(b_, t_))
                stage_tiles[i] = st

            for i in range(prefetch):
                issue_load(i)

            # b3 transposes (after the prologue so their ACT quants don't
            # head-of-line block the scalar-ring prefetch triggers):
            # units u = 2s + r (m = 256s + 2c + r)
            b3_views = {
                (s, r): xb3_tiles[s][:, r, :].rearrange(
                    "p (k two) -> p two k", two=2
                )
                for s in range(2) for r in range(2)
            }
            for t in range(KP):
                for parp in range(2):
                    xpose_group(
                        BL - 1, t, parp,
                        [(b3_views[(s, r)], 2 * s + r)
                         for s in range(2) for r in range(2)],
                    )
            trps.release()
            mmps = tc.alloc_tile_pool(name="mmps", bufs=4, space="PSUM")

            wq_all = {}
            for i, (b_, t_) in enumerate(flat):
                wqt = wqp.tile([P, 2, N], FP8, name="wq", tag="wq")
                nc.vector.tensor_scalar(
                    wqt[:], stage_tiles.pop(i)[:], sw_ap, None,
                    op0=mybir.AluOpType.mult,
                )
                if i + prefetch < len(flat):
                    issue_load(i + prefetch)
                wq_all[(b_, t_)] = wqt

                if t_ == KP - 1:
                    b = b_
                    if with_bias:
                        b1 = bias1p.tile([1, N], BF16, name="b1", tag="b1")
                        nc.gpsimd.dma_start(b1[:], bias[b, :, :])
                        bb = biasbp.tile([P, N], BF16, name="bb", tag="bb")
                        nc.gpsimd.partition_broadcast(bb[:], b1[:])

                    wq_tiles = [wq_all.pop((b, t)) for t in range(KP)]
                    for mh in range(MT // 2):
                        ost2 = ostp.tile([P, 2, N], BF16, name="ost", tag="ost")
                        # 2-bank-wide psum tiles halve the per-instruction
                        # overhead of the drains (~1.2us fixed cost each)
                        psums = [
                            [
                                mmps.tile([P, 2 * NFREE], F32,
                                          name=f"mm{mi}{h}", tag="mm")
                                for h in range(NT // 2)
                            ]
                            for mi in range(2)
                        ]
                        for t in range(KP):
                            for mi in range(2):
                                u = 2 * mh + mi
                                lhsT = xqt[:, t, :,
                                           b * M + u * P:b * M + (u + 1) * P]
                                for nt in range(NT):
                                    ps = psums[mi][nt // 2]
                                    lo = (nt % 2) * NFREE
                                    nc.tensor.matmul(
                                        ps[:, lo:lo + NFREE],
                                        lhsT,
                                        wq_tiles[t][:, :,
                                                    nt * NFREE:(nt + 1) * NFREE],
                                        start=(t == 0),
                                        stop=(t == KP - 1),
                                        perf_mode=mybir.MatmulPerfMode.DoubleRow,
                                    )
                        for mi in range(2):
                            for h in range(NT // 2):
                                o_ap = ost2[:, mi,
                                            h * 2 * NFREE:(h + 1) * 2 * NFREE]
                                if with_bias:
                                    nc.vector.scalar_tensor_tensor(
                                        o_ap, psums[mi][h][:], c_ap,
                                        bb[:, h * 2 * NFREE:(h + 1) * 2 * NFREE],
                                        op0=mybir.AluOpType.mult,
                                        op1=mybir.AluOpType.add,
                                    )
                                elif b == BL - 1:
                                    # last batch's drains on the idle ACT
                                    # engine: its ~25us post-stream tail is
                                    # the kernel's critical path
                                    nc.scalar.activation(
                                        o_ap, psums[mi][h][:],
                                        mybir.ActivationFunctionType.Copy,
                                        scale=c_ap,
                                    )
                                else:
                                    nc.vector.tensor_scalar(
                                        o_ap, psums[mi][h][:], c_ap, None,
                                        op0=mybir.AluOpType.mult,
                                    )
                        dst = out[b, 2 * mh * P:(2 * mh + 2) * P, :]
                        if b < XRES:
                            dst = dst.rearrange("(p k) n -> k p n", p=2)
                        else:
                            # b3's m-rows are pair-packed: m = 256*mh + 2c + r
                            dst = dst.rearrange("(p r) n -> p r n", r=2)
                        nc.gpsimd.dma_start(dst, ost2[:])

            mmps.release()
            if with_bias:
                biasbp.release()
                bias1p.release()
            ostp.release()
            wqp.release()
            restage.release()

    nc.compile()
    return nc


def _get_nc(with_bias):
    key = "fused_b" if with_bias else "fused_nb"
    if key not in _cache:
        _cache[key] = _build_fused_nc(with_bias)
    return _cache[key]


# test.py introspection: exec times (ns) of the last kernel() call.
last_run_info = {}


def kernel(input, weight, bias, _profile=False, _repeat=1, _trace_kwargs=None):
    input = np.ascontiguousarray(input, dtype=np.float32)
    weight = np.ascontiguousarray(weight, dtype=np.float32)
    bias = np.ascontiguousarray(bias, dtype=np.float32)
    assert input.shape == (B, M, K) and weight.shape == (B, K, N)
    assert bias.shape == (B, 1, N)

    consts = np.array([[FP8_HALF_MAX, 1.0]], dtype=np.float32)
    in_maps = [
        {
            "x": input[c * BL:(c + 1) * BL],
            "w": weight[c * BL:(c + 1) * BL],
            "bias": bias[c * BL:(c + 1) * BL],
            "consts": consts,
        }
        for c in range(NCORES)
    ]

    kw = dict(trace=_profile)
    if _trace_kwargs:
        kw.update(_trace_kwargs)

    # bias is exactly zero in this workload; the no-bias NEFF skips the
    # broadcast-add (drains become scaled copies, ACT-assisted at the tail).
    # The with-bias NEFF stays available for correctness on any input.
    nc = _get_nc(with_bias=bool(np.any(bias)))
    times = []
    res = None
    for _ in range(max(1, _repeat)):
        res = run_bass_kernel_spmd(nc, in_maps, core_ids=list(range(NCORES)), **kw)
        times.append(res.exec_time_ns)

    last_run_info.clear()
    last_run_info["amax_times"] = None
    last_run_info["mm_times"] = times
    last_run_info["amax_exec_ns"] = None
    last_run_info["mm_exec_ns"] = min(t for t in times if t) if any(times) else None
    last_run_info["mm_results"] = res

    out = np.concatenate(
        [np.asarray(res.results[c]["out"]).astype(np.float32) for c in range(NCORES)],
        axis=0,
    )
    return out



# revision 6
# speedup vs baseline: 1.0250x; 1.0250x over previous
"""FP8 batch-matmul-dense kernel for Trainium2 (8 NeuronCores, batch-sharded).

Problem: out[b] = fp8qdq(x)[b] @ fp8qdq(w)[b] + bias[b]
  x: [32, 512, 2048] f32, w: [32, 2048, 2048] f32, bias: [32, 1, 2048] f32
  fp8qdq = torchao-style dynamic tensorwise scaling: s = 448/amax(|t|),
  q = e4m3fn(t*s), dq = q/s. Global (whole-tensor) amax.

Sharding: batch axis across 8 cores, 4 slices each (expert-parallel style).

v3 design (single fused NEFF):
  Phase A streams x then w at fp32, computing exact local amaxes on DVE;
  amax_x / amax_w are AllReduce(max)'d (a dummy warmup AllReduce pays the
  first-collective setup under the x loads). x is PE-transposed as it
  arrives and drained to a RAW fp16 xT (8MiB, no scale needed) so the
  transposes never gate on the ARx result; once sx lands, ACT quantizes
  xT -> 4MiB resident fp8 lhsT codes and xT's space is recycled. The tail
  of the w stream (last RETAIN row-pair tiles in stream order) is
  ACT-downcast to resident fp16 (1MiB/tile), cutting the phase-B re-read
  by 2MiB/tile; the stream order is permuted so the retained set spreads
  across batches b1..b3, balancing phase-B DMA per batch against the PE.
  Phase B re-reads only the non-retained w, quantizes on DVE (fp32 for
  re-read tiles, 2x-rate fp16 for retained), and runs DoubleRow fp8
  matmuls (fp32 PSUM accum) in mt-pair sweeps over 8 PSUM banks, drains
  bias+rescale to bf16 and stores via SWDGE (host upcasts).

Performance model (from ntff profiling):
  - The 16 SDMA engines (~22GB/s each on 8-16KB descriptors) bind phase A
    (80MiB: 16 x + 64 w) and roughly tie the PE in phase B (re-read
    50MiB + 8 out vs ~160us of DoubleRow matmul). All tiles move as
    [128, 2, N] row-pairs (one 16KB-contiguous descriptor per partition).
  - Engine queues are strict FIFO: all load triggers ride the sync HWDGE
    ring; the scalar (ACT) queue holds only the x drains / xqt quants /
    retention downcasts, each gated strictly later than the last, so
    nothing head-of-line blocks. sx math sits SX_DEPTH w-reduces deep in
    the DVE FIFO so DVE reaches it just as the ARx result lands.
  - The ARw collective (~40us against a busy SDMA path) is covered by a
    4-deep re-read prefetch prologue into the freed stage slots.

Quantization math (matches the reference lattice exactly): s' = 224/amax
  (= fl(448/amax)/2 exactly) because TRN fp8_e4m3 tops out at 240, not
  448: the OCP e4m3fn lattice scaled by 1/2 lands exactly on the TRN
  lattice. Matmul runs on raw fp8 codes (exact products, fp32 PSUM
  accum); output is rescaled by c = 1/(sx'*sw'). x codes pass through a
  raw fp16 intermediate and retained w tiles are quantized from fp16:
  the extra 2^-11 rounding flips ~0.8% of codes by 1 ulp, adding ~1e-2
  of the 2e-2 relative budget (measured: comfortably inside the gate).

Per-core HBM traffic: 16 (x) + 64 (w) + 50 (w re-read) + 8 (out bf16)
= 138MiB, one NEFF ramp.
"""

import os
import sys

for _p in ("/root/.axon_site", "/root/.axon_site/_ro/trn_rl_repo", "/opt/trn_rl_repo"):
    if os.path.isdir(_p) and _p not in sys.path:
        sys.path.append(_p)

import numpy as np

import concourse.bass as bass
import concourse.bass_isa as bass_isa
import concourse.mybir as mybir
import concourse.tile as tile
from concourse import bacc
from concourse.bass_utils import run_bass_kernel_spmd
from concourse.masks import make_identity

# Problem shape (hardcoded per contest rules).
B, M, K, N = 32, 512, 2048, 2048
NCORES = 8
BL = B // NCORES          # 4 batch slices per core
P = 128
KT = K // P               # 16 k-tiles per batch
KP = KT // 2              # 8 k-groups (256 rows, row-pair packed) per batch
MT = M // P               # 4 m-tiles
NFREE = 512               # matmul moving free dim (one PSUM bank)
NT = N // NFREE           # 4 n-tiles
SX_DEPTH = 17             # staged (2MiB) w reduces before sx in the DVE FIFO
RETAIN = 7                # w k-group tiles retained as fp16 (with_bias: -2)
PREFETCH = 4              # phase-B re-read loads in flight before 1st quant
FP8_HALF_MAX = 224.0      # 448/2: OCP grid mapped onto TRN e4m3

F32 = mybir.dt.float32
F16 = mybir.dt.float16
BF16 = mybir.dt.bfloat16
FP8 = mybir.dt.float8e4

_cache = {}


def _build_fused_nc(with_bias=True):
    nc = bacc.Bacc("TRN2", target_bir_lowering=False, debug=False, num_devices=NCORES)
    x = nc.dram_tensor("x", [BL, M, K], F32, kind="ExternalInput")
    w = nc.dram_tensor("w", [BL, K, N], F32, kind="ExternalInput")
    bias = nc.dram_tensor("bias", [BL, 1, N], F32, kind="ExternalInput")
    consts = nc.dram_tensor("consts", [1, 2], F32, kind="ExternalInput")
    out = nc.dram_tensor("out", [BL, M, N], BF16, kind="ExternalOutput")

    rg = [list(range(NCORES))]
    retain = RETAIN if with_bias is False else RETAIN - 2

    # w stream order: natural order with the retained set moved to the
    # end so retention only needs SBUF after the xT space frees. The
    # retained set spreads over b1..b3 to even phase-B DMA per batch.
    flat = [(b, t) for b in range(BL) for t in range(KP)]
    ret_set = [(1, 7), (2, 6), (2, 7), (3, 4), (3, 5), (3, 6), (3, 7)][-retain:]
    stream_plan = [bt for bt in flat if bt not in ret_set] + ret_set

    def w_pair_src(b, t):
        """w[b] rows [256t, 256t+256) as [128, 2, N]: partition p holds DRAM
        rows 2p/2p+1 -> one 16KB-contiguous descriptor per partition."""
        return w[b, t * 2 * P:(t + 1) * 2 * P, :].rearrange(
            "(p r) n -> p r n", r=2
        )

    def x_pair_src(b, s):
        """x[b] rows [256s, 256s+256) as [128, 2, K] row-pair tiles."""
        return x[b, s * 2 * P:(s + 1) * 2 * P, :].rearrange(
            "(p r) n -> p r n", r=2
        )

    with tile.TileContext(nc) as tc:
        with (
            tc.tile_pool(name="small", bufs=1) as small,
            tc.tile_pool(name="acc", bufs=1) as accp,
            tc.tile_pool(name="xqt", bufs=1) as xqtp,
            tc.tile_pool(name="wstage", bufs=2) as wstage,
            tc.tile_pool(name="dram", bufs=6, space="DRAM") as dram,
        ):
            ident = small.tile([P, P], F32, name="ident")
            make_identity(nc, ident[:])
            cst = small.tile([1, 2], F32, name="cst")
            nc.sync.dma_start(cst[:], consts[0:1, :])
            # scl slots: 0=1/ax, 1=sx, 2=1/aw, 3=sw, 4=sx*sw, 5=c
            scl = small.tile([1, 8], F32, name="scl")
            axg = small.tile([1, 1], F32, name="axg")
            awg = small.tile([1, 1], F32, name="awg")
            cb = small.tile([P, 4], F32, name="cb")   # 0=sx, 1=sw, 2=c

            acc = accp.tile([P, 8 + BL * KP], F32, name="acc")
            red = accp.tile([P, 2], F32, name="red")
            par = accp.tile([P, 2], F32, name="par")

            # resident fp8 lhsT codes: [ki, t, par, b*M + u*128 + c] where
            # (ki, par) pair k = 256t + 2*ki + par (matching the w pairing)
            # and unit u = 2s + r pairs m = 256s + 2c + r.
            xqt = xqtp.tile([P, KP, 2, BL * M], FP8, name="xqt")

            dum_in = dram.tile([1, 8], F32, name="dum_in")
            dum_out = dram.tile([1, 8], F32, name="dum_out")
            arx_in = dram.tile([1, 8], F32, name="arx_in")
            arx_out = dram.tile([1, 8], F32, name="arx_out")
            arw_in = dram.tile([1, 8], F32, name="arw_in")
            arw_out = dram.tile([1, 8], F32, name="arw_out")

            # warmup collective: pays the ~80us first-collective setup while
            # the x loads stream.
            nc.gpsimd.dma_start(dum_in[0:1, 0:2], cst[:])
            nc.gpsimd.collective_compute(
                "AllReduce", mybir.AluOpType.max, replica_groups=rg,
                ins=[dum_in.opt()], outs=[dum_out.opt()],
            )

            xtp = tc.alloc_tile_pool(name="xt", bufs=BL)
            xstage = tc.alloc_tile_pool(name="xstage", bufs=4)
            trps = tc.alloc_tile_pool(name="trps", bufs=4, space="PSUM")

            # ---- x: stream, amax, PE-transpose, drain raw fp16 xT ----
            xts = []
            for b in range(BL):
                xs = []
                for s in range(2):
                    st = xstage.tile([P, 2, K], F32, name="xs", tag="xs")
                    nc.sync.dma_start(st[:], x_pair_src(b, s))
                    nc.vector.tensor_reduce(
                        acc[:, 2 * b + s:2 * b + s + 1], st[:],
                        axis=mybir.AxisListType.XY, op=mybir.AluOpType.max,
                        apply_absolute_value=True,
                    )
                    xs.append(st)
                xt = xtp.tile([P, KP, 2, M], F16, name="xt", tag="xt")
                views = {
                    (s, r): xs[s][:, r, :].rearrange("p (k two) -> p two k", two=2)
                    for s in range(2) for r in range(2)
                }
                for t in range(KP):
                    for parp in range(2):
                        ps = trps.tile([P, M], F32, name="tps", tag="tps")
                        for s in range(2):
                            for r in range(2):
                                u = 2 * s + r
                                nc.tensor.transpose(
                                    ps[:, u * P:(u + 1) * P],
                                    views[(s, r)][:, parp, t * P:(t + 1) * P],
                                    ident[:],
                                )
                        nc.scalar.activation(
                            xt[:, t, parp, :], ps[:],
                            mybir.ActivationFunctionType.Copy,
                        )
                xts.append(xt)

            # ---- amax_x AllReduce trigger (result consumed later) ----
            nc.vector.tensor_reduce(
                red[:, 0:1], acc[:, 0:2 * BL],
                axis=mybir.AxisListType.X, op=mybir.AluOpType.max,
            )
            nc.gpsimd.partition_all_reduce(
                par[:, 0:1], red[:, 0:1], channels=P,
                reduce_op=bass_isa.ReduceOp.max,
            )
            nc.gpsimd.dma_start(arx_in[0:1, 0:1], par[0:1, 0:1])
            nc.gpsimd.collective_compute(
                "AllReduce", mybir.AluOpType.max, replica_groups=rg,
                ins=[arx_in.opt()], outs=[arx_out.opt()],
            )
            nc.gpsimd.dma_start(axg[:], arx_out[0:1, 0:1])

            trps.release()
            xstage.release()

            col = [8]
            wret = {}
            wretp = [None]

            def stage_w_load(bt):
                ws = wstage.tile([P, 2, N], F32, name="ws", tag="ws")
                nc.sync.dma_start(ws[:], w_pair_src(*bt))
                nc.vector.tensor_reduce(
                    acc[:, col[0]:col[0] + 1], ws[:],
                    axis=mybir.AxisListType.XY, op=mybir.AluOpType.max,
                    apply_absolute_value=True,
                )
                col[0] += 1
                if bt in ret_set:
                    wr = wretp[0].tile([P, 2, N], F16, name="wr", tag="wr")
                    nc.scalar.activation(
                        wr[:], ws[:], mybir.ActivationFunctionType.Copy,
                    )
                    wret[bt] = wr

            for bt in stream_plan[:SX_DEPTH]:
                stage_w_load(bt)

            # sx = 224 / max(amax_x, 1e-12): DVE reaches this ~17 staged
            # reduces deep, by when the AllReduce result has landed.
            nc.vector.tensor_scalar_max(axg[:], axg[:], 1e-12)
            nc.vector.reciprocal(scl[0:1, 0:1], axg[:])
            nc.vector.tensor_scalar_mul(scl[0:1, 1:2], scl[0:1, 0:1], FP8_HALF_MAX)
            nc.gpsimd.partition_broadcast(cb[:, 0:1], scl[0:1, 1:2])
            sx_ap = cb[:, 0:1]

            # xqt quants ride the ACT queue (free once the x drains end);
            # xT's 8MiB then recycles into the w retention pool.
            for b in range(BL):
                nc.scalar.activation(
                    xqt[:, :, :, b * M:(b + 1) * M], xts[b][:],
                    mybir.ActivationFunctionType.Copy, scale=sx_ap,
                )
            xtp.release()
            wretp[0] = tc.alloc_tile_pool(name="wret", bufs=max(retain, 1))

            for bt in stream_plan[SX_DEPTH:]:
                stage_w_load(bt)

            # ---- amax_w AllReduce ----
            nc.vector.tensor_reduce(
                red[:, 1:2], acc[:, 8:col[0]],
                axis=mybir.AxisListType.X, op=mybir.AluOpType.max,
            )
            nc.gpsimd.partition_all_reduce(
                par[:, 1:2], red[:, 1:2], channels=P,
                reduce_op=bass_isa.ReduceOp.max,
            )
            nc.gpsimd.dma_start(arw_in[0:1, 0:1], par[0:1, 1:2])
            nc.gpsimd.collective_compute(
                "AllReduce", mybir.AluOpType.max, replica_groups=rg,
                ins=[arw_in.opt()], outs=[arw_out.opt()],
            )
            nc.gpsimd.dma_start(awg[:], arw_out[0:1, 0:1])
            # sw = 224 / max(amax_w, 1e-12); c = 1/(sx*sw)
            nc.vector.tensor_scalar_max(awg[:], awg[:], 1e-12)
            nc.vector.reciprocal(scl[0:1, 2:3], awg[:])
            nc.vector.tensor_scalar_mul(scl[0:1, 3:4], scl[0:1, 2:3], FP8_HALF_MAX)
            nc.vector.tensor_tensor(
                scl[0:1, 4:5], scl[0:1, 1:2], scl[0:1, 3:4],
                mybir.AluOpType.mult,
            )
            nc.vector.reciprocal(scl[0:1, 5:6], scl[0:1, 4:5])
            nc.gpsimd.partition_broadcast(cb[:, 1:2], scl[0:1, 3:4])
            nc.gpsimd.partition_broadcast(cb[:, 2:3], scl[0:1, 5:6])
            sw_ap = cb[:, 1:2]
            c_ap = cb[:, 2:3]

            # ---- phase B: software-pipelined re-read + quantize + mm ----
            restage = tc.alloc_tile_pool(name="restage", bufs=2)
            wqp = tc.alloc_tile_pool(name="wq", bufs=9)
            ostp = tc.alloc_tile_pool(name="ost", bufs=2)
            if with_bias:
                bias1p = tc.alloc_tile_pool(name="bias1", bufs=1)
                biasbp = tc.alloc_tile_pool(name="biasb", bufs=2)

            reread_plan = [bt for bt in flat if bt not in ret_set]
            stage_tiles = {}
            nload = [0]

            def issue_load():
                if nload[0] >= len(reread_plan):
                    return
                bt = reread_plan[nload[0]]
                pool = restage if nload[0] % 2 == 1 else wstage
                st = pool.tile([P, 2, N], F32, name="ws", tag="ws")
                nc.sync.dma_start(st[:], w_pair_src(*bt))
                stage_tiles[bt] = st
                nload[0] += 1

            for _ in range(PREFETCH):
                issue_load()

            mmps = tc.alloc_tile_pool(name="mmps", bufs=4, space="PSUM")

            wq_all = {}
            for b_, t_ in flat:
                wqt = wqp.tile([P, 2, N], FP8, name="wq", tag="wq")
                if (b_, t_) in ret_set:
                    nc.vector.tensor_scalar(
                        wqt[:], wret[(b_, t_)][:], sw_ap, None,
                        op0=mybir.AluOpType.mult,
                    )
                else:
                    nc.vector.tensor_scalar(
                        wqt[:], stage_tiles.pop((b_, t_))[:], sw_ap, None,
                        op0=mybir.AluOpType.mult,
                    )
                    issue_load()
                wq_all[(b_, t_)] = wqt

                if t_ == KP - 1:
                    b = b_
                    if with_bias:
                        b1 = bias1p.tile([1, N], BF16, name="b1", tag="b1")
                        nc.gpsimd.dma_start(b1[:], bias[b, :, :])
                        bb = biasbp.tile([P, N], BF16, name="bb", tag="bb")
                        nc.gpsimd.partition_broadcast(bb[:], b1[:])

                    wq_tiles = [wq_all.pop((b, t)) for t in range(KP)]
                    for mh in range(MT // 2):
                        ost2 = ostp.tile([P, 2, N], BF16, name="ost", tag="ost")
                        # 2-bank-wide psum tiles halve the per-instruction
                        # overhead of the drains (~1.2us fixed cost each)
                        psums = [
                            [
                                mmps.tile([P, 2 * NFREE], F32,
                                          name=f"mm{mi}{h}", tag="mm")
                                for h in range(NT // 2)
                            ]
                            for mi in range(2)
                        ]
                        for t in range(KP):
                            for mi in range(2):
                                u = 2 * mh + mi
                                lhsT = xqt[:, t, :,
                                           b * M + u * P:b * M + (u + 1) * P]
                                for nt in range(NT):
                                    ps = psums[mi][nt // 2]
                                    lo = (nt % 2) * NFREE
                                    nc.tensor.matmul(
                                        ps[:, lo:lo + NFREE],
                                        lhsT,
                                        wq_tiles[t][:, :,
                                                    nt * NFREE:(nt + 1) * NFREE],
                                        start=(t == 0),
                                        stop=(t == KP - 1),
                                        perf_mode=mybir.MatmulPerfMode.DoubleRow,
                                    )
                        for mi in range(2):
                            for h in range(NT // 2):
                                o_ap = ost2[:, mi,
                                            h * 2 * NFREE:(h + 1) * 2 * NFREE]
                                if with_bias:
                                    nc.vector.scalar_tensor_tensor(
                                        o_ap, psums[mi][h][:], c_ap,
                                        bb[:, h * 2 * NFREE:(h + 1) * 2 * NFREE],
                                        op0=mybir.AluOpType.mult,
                                        op1=mybir.AluOpType.add,
                                    )
                                elif b == BL - 1:
                                    # last batch's drains on the idle ACT
                                    # engine: its post-stream tail is the
                                    # kernel's critical path
                                    nc.scalar.activation(
                                        o_ap, psums[mi][h][:],
                                        mybir.ActivationFunctionType.Copy,
                                        scale=c_ap,
                                    )
                                else:
                                    nc.vector.tensor_scalar(
                                        o_ap, psums[mi][h][:], c_ap, None,
                                        op0=mybir.AluOpType.mult,
                                    )
                        # m-rows are pair-packed: m = 256*mh + 2c + r
                        dst = out[b, 2 * mh * P:(2 * mh + 2) * P, :].rearrange(
                            "(p r) n -> p r n", r=2
                        )
                        nc.gpsimd.dma_start(dst, ost2[:])

            mmps.release()
            if with_bias:
                biasbp.release()
                bias1p.release()
            ostp.release()
            wqp.release()
            restage.release()
            wretp[0].release()

    nc.compile()
    return nc


def _get_nc(with_bias):
    key = "fused_b" if with_bias else "fused_nb"
    if key not in _cache:
        _cache[key] = _build_fused_nc(with_bias)
    return _cache[key]


# test.py introspection: exec times (ns) of the last kernel() call.
last_run_info = {}


def kernel(input, weight, bias, _profile=False, _repeat=1, _trace_kwargs=None):
    input = np.ascontiguousarray(input, dtype=np.float32)
    weight = np.ascontiguousarray(weight, dtype=np.float32)
    bias = np.ascontiguousarray(bias, dtype=np.float32)
    assert input.shape == (B, M, K) and weight.shape == (B, K, N)
    assert bias.shape == (B, 1, N)

    consts = np.array([[FP8_HALF_MAX, 1.0]], dtype=np.float32)
    in_maps = [
        {
            "x": input[c * BL:(c + 1) * BL],
            "w": weight[c * BL:(c + 1) * BL],
            "bias": bias[c * BL:(c + 1) * BL],
            "consts": consts,
        }
        for c in range(NCORES)
    ]

    kw = dict(trace=_profile)
    if _trace_kwargs:
        kw.update(_trace_kwargs)

    # bias is exactly zero in this workload; the no-bias NEFF skips the
    # broadcast-add (drains become scaled copies, ACT-assisted at the tail).
    # The with-bias NEFF stays available for correctness on any input.
    nc = _get_nc(with_bias=bool(np.any(bias)))
    times = []
    res = None
    for _ in range(max(1, _repeat)):
        res = run_bass_kernel_spmd(nc, in_maps, core_ids=list(range(NCORES)), **kw)
        times.append(res.exec_time_ns)

    last_run_info.clear()
    last_run_info["amax_times"] = None
    last_run_info["mm_times"] = times
    last_run_info["amax_exec_ns"] = None
    last_run_info["mm_exec_ns"] = min(t for t in times if t) if any(times) else None
    last_run_info["mm_results"] = res

    out = np.concatenate(
        [np.asarray(res.results[c]["out"]).astype(np.float32) for c in range(NCORES)],
        axis=0,
    )
    return out


# revision 20
# speedup vs baseline: 1.0712x; 1.0451x over previous
"""FP8 batch-matmul-dense kernel for Trainium2 (8 NeuronCores, batch-sharded).

Problem: out[b] = fp8qdq(x)[b] @ fp8qdq(w)[b] + bias[b]
  x: [32, 512, 2048] f32, w: [32, 2048, 2048] f32, bias: [32, 1, 2048] f32
  fp8qdq = torchao-style dynamic tensorwise scaling: s = 448/amax(|t|),
  q = e4m3fn(t*s), dq = q/s. Global (whole-tensor) amax.

Sharding: batch axis across 8 cores, 4 slices each (expert-parallel style).

v3 design (single fused NEFF):
  Phase A streams x then w at fp32, computing exact local amaxes on DVE;
  amax_x / amax_w are AllReduce(max)'d (a dummy warmup AllReduce pays the
  first-collective setup under the x loads). x is PE-transposed as it
  arrives and drained to a RAW fp16 xT (8MiB, no scale needed) so the
  transposes never gate on the ARx result; once sx lands, ACT quantizes
  xT -> 4MiB resident fp8 lhsT codes and xT's space is recycled. The tail
  of the w stream (last RETAIN row-pair tiles in stream order) is
  ACT-downcast to resident fp16 (1MiB/tile), cutting the phase-B re-read
  by 2MiB/tile; the stream order is permuted so the retained set spreads
  across batches b1..b3, balancing phase-B DMA per batch against the PE.
  Phase B re-reads only the non-retained w, quantizes on DVE (fp32 for
  re-read tiles, 2x-rate fp16 for retained), and runs DoubleRow fp8
  matmuls (fp32 PSUM accum) in mt-pair sweeps over 8 PSUM banks, drains
  bias+rescale to bf16 and stores via SWDGE (host upcasts).

Performance model (from ntff profiling):
  - The 16 SDMA engines (~22GB/s each on 8-16KB descriptors) bind phase A
    (80MiB: 16 x + 64 w) and roughly tie the PE in phase B (re-read
    50MiB + 8 out vs ~160us of DoubleRow matmul). All tiles move as
    [128, 2, N] row-pairs (one 16KB-contiguous descriptor per partition).
  - Engine queues are strict FIFO: all load triggers ride the sync HWDGE
    ring; the scalar (ACT) queue holds only the x drains / xqt quants /
    retention downcasts, each gated strictly later than the last, so
    nothing head-of-line blocks. sx math sits SX_DEPTH w-reduces deep in
    the DVE FIFO so DVE reaches it just as the ARx result lands.
  - The ARw collective (~40us against a busy SDMA path) is covered by a
    4-deep re-read prefetch prologue into the freed stage slots.

Quantization math (matches the reference lattice exactly): s' = 224/amax
  (= fl(448/amax)/2 exactly) because TRN fp8_e4m3 tops out at 240, not
  448: the OCP e4m3fn lattice scaled by 1/2 lands exactly on the TRN
  lattice. Matmul runs on raw fp8 codes (exact products, fp32 PSUM
  accum); output is rescaled by c = 1/(sx'*sw'). x codes pass through a
  raw fp16 intermediate and retained w tiles are quantized from fp16:
  the extra 2^-11 rounding flips ~0.8% of codes by 1 ulp, adding ~1e-2
  of the 2e-2 relative budget (measured: comfortably inside the gate).

Per-core HBM traffic: 16 (x) + 64 (w) + 50 (w re-read) + 8 (out bf16)
= 138MiB, one NEFF ramp.
"""

import os
import sys

for _p in ("/root/.axon_site", "/root/.axon_site/_ro/trn_rl_repo", "/opt/trn_rl_repo"):
    if os.path.isdir(_p) and _p not in sys.path:
        sys.path.append(_p)

import numpy as np

import concourse.bass as bass
import concourse.bass_isa as bass_isa
import concourse.mybir as mybir
import concourse.tile as tile
from concourse import bacc
from concourse.bass_utils import run_bass_kernel_spmd
from concourse.masks import make_identity

# Problem shape (hardcoded per contest rules).
B, M, K, N = 32, 512, 2048, 2048
NCORES = 8
BL = B // NCORES          # 4 batch slices per core
P = 128
KT = K // P               # 16 k-tiles per batch
KP = KT // 2              # 8 k-groups (256 rows, row-pair packed) per batch
MT = M // P               # 4 m-tiles
NFREE = 512               # matmul moving free dim (one PSUM bank)
NT = N // NFREE           # 4 n-tiles
SX_DEPTH = 15             # staged (2MiB) w reduces before sx in the DVE FIFO
RETAIN = 7                # w k-group tiles retained as fp16 (with_bias: -2)
PREFETCH = 3              # phase-B re-read loads in flight before 1st quant
FP8_HALF_MAX = 224.0      # 448/2: OCP grid mapped onto TRN e4m3

F32 = mybir.dt.float32
F16 = mybir.dt.float16
BF16 = mybir.dt.bfloat16
FP8 = mybir.dt.float8e4

_cache = {}


def _build_fused_nc(with_bias=True):
    nc = bacc.Bacc("TRN2", target_bir_lowering=False, debug=False, num_devices=NCORES)
    x = nc.dram_tensor("x", [BL, M, K], F32, kind="ExternalInput")
    w = nc.dram_tensor("w", [BL, K, N], F32, kind="ExternalInput")
    bias = nc.dram_tensor("bias", [BL, 1, N], F32, kind="ExternalInput")
    consts = nc.dram_tensor("consts", [1, 2], F32, kind="ExternalInput")
    out = nc.dram_tensor("out", [BL, M, N], BF16, kind="ExternalOutput")

    rg = [list(range(NCORES))]
    retain = RETAIN if with_bias is False else RETAIN - 2

    # w stream order: natural order with the retained set moved to the
    # end so retention only needs SBUF after the xT space frees. The
    # retained set spreads over b1..b3 to even phase-B DMA per batch.
    flat = [(b, t) for b in range(BL) for t in range(KP)]
    ret_set = [(1, 7), (2, 6), (2, 7),
               (3, 4), (3, 5), (3, 6), (3, 7)][-retain:]
    stream_plan = [bt for bt in flat if bt not in ret_set] + ret_set

    def w_pair_src(b, t):
        """w[b] rows [256t, 256t+256) as [128, 2, N]: partition p holds DRAM
        rows 2p/2p+1 -> one 16KB-contiguous descriptor per partition."""
        return w[b, t * 2 * P:(t + 1) * 2 * P, :].rearrange(
            "(p r) n -> p r n", r=2
        )

    def x_half_src(b, s):
        """x[b] rows [256s, 256s+256) as [128, 2, K]: partition p holds
        rows {256s + p, 256s + 128 + p} (plain m-blocks u = 2s, 2s+1)."""
        return x[b, s * 2 * P:(s + 1) * 2 * P, :].rearrange(
            "(u p) n -> p u n", u=2
        )

    with tile.TileContext(nc) as tc:
        with (
            tc.tile_pool(name="small", bufs=1) as small,
            tc.tile_pool(name="acc", bufs=1) as accp,
            tc.tile_pool(name="xqt", bufs=1) as xqtp,
            tc.tile_pool(name="wstage", bufs=3) as wstage,
            tc.tile_pool(name="dram", bufs=6, space="DRAM") as dram,
        ):
            ident = small.tile([P, P], F32, name="ident")
            make_identity(nc, ident[:])
            cst = small.tile([1, 2], F32, name="cst")
            nc.sync.dma_start(cst[:], consts[0:1, :])
            # scl slots: 0=1/ax, 1=sx, 2=1/aw, 3=sw, 4=sx*sw, 5=c
            scl = small.tile([1, 8], F32, name="scl")
            axg = small.tile([1, 1], F32, name="axg")
            awg = small.tile([1, 1], F32, name="awg")
            cb = small.tile([P, 4], F32, name="cb")   # 0=sx, 1=sw, 2=c

            acc = accp.tile([P, 8 + BL * KP], F32, name="acc")
            red = accp.tile([P, 2], F32, name="red")
            par = accp.tile([P, 2], F32, name="par")

            # resident fp8 lhsT codes: [ki, t, par, b*M + u*128 + c] where
            # (ki, par) pair k = 256t + 2*ki + par (matching the w pairing)
            # and unit u = 2s + r pairs m = 256s + 2c + r.
            xqt = xqtp.tile([P, KP, 2, BL * M], FP8, name="xqt")

            dum_in = dram.tile([1, 8], F32, name="dum_in")
            dum_out = dram.tile([1, 8], F32, name="dum_out")
            arx_in = dram.tile([1, 8], F32, name="arx_in")
            arx_out = dram.tile([1, 8], F32, name="arx_out")
            arw_in = dram.tile([1, 8], F32, name="arw_in")
            arw_out = dram.tile([1, 8], F32, name="arw_out")

            # warmup collective: pays the ~80us first-collective setup while
            # the x loads stream.
            nc.gpsimd.dma_start(dum_in[0:1, 0:2], cst[:])
            nc.gpsimd.collective_compute(
                "AllReduce", mybir.AluOpType.max, replica_groups=rg,
                ins=[dum_in.opt()], outs=[dum_out.opt()],
            )

            xtp = tc.alloc_tile_pool(name="xt", bufs=BL)
            xstage = tc.alloc_tile_pool(name="xstage", bufs=3)
            trps = tc.alloc_tile_pool(name="trps", bufs=3, space="PSUM")

            # ---- x: stream, amax, PE-transpose, drain raw fp16 xT ----
            # x rides the sync ring only: the x stream is transpose-paced
            # (~12us/batch), slower than even a single HWDGE ring.
            xts = []
            for b in range(BL):
                views = {}
                for s in range(2):
                    st = xstage.tile([P, 2, K], F32, name="xs", tag="xs")
                    nc.sync.dma_start(st[:], x_half_src(b, s))
                    nc.vector.tensor_reduce(
                        acc[:, 2 * b + s:2 * b + s + 1], st[:],
                        axis=mybir.AxisListType.XY, op=mybir.AluOpType.max,
                        apply_absolute_value=True,
                    )
                    for j in range(2):
                        views[2 * s + j] = st[:, j, :].rearrange(
                            "p (k two) -> p two k", two=2
                        )
                xt = xtp.tile([P, KP, 2, M], F16, name="xt", tag="xt")
                for t in range(KP):
                    # one [P, 1024] psum per k-group: cols parp*512 + u*128
                    # + c, drained fused to xt[:, t, :, :] (fp16, no scale)
                    ps = trps.tile([P, 2 * M], F32, name="tps", tag="tps")
                    for parp in range(2):
                        for u in range(MT):
                            nc.tensor.transpose(
                                ps[:, parp * M + u * P:parp * M + (u + 1) * P],
                                views[u][:, parp, t * P:(t + 1) * P],
                                ident[:],
                            )
                    nc.scalar.activation(
                        xt[:, t, :, :], ps[:],
                        mybir.ActivationFunctionType.Copy,
                    )
                xts.append(xt)

            # ---- amax_x AllReduce trigger (result consumed later) ----
            nc.vector.tensor_reduce(
                red[:, 0:1], acc[:, 0:2 * BL],
                axis=mybir.AxisListType.X, op=mybir.AluOpType.max,
            )
            nc.gpsimd.partition_all_reduce(
                par[:, 0:1], red[:, 0:1], channels=P,
                reduce_op=bass_isa.ReduceOp.max,
            )
            nc.gpsimd.dma_start(arx_in[0:1, 0:1], par[0:1, 0:1])
            nc.gpsimd.collective_compute(
                "AllReduce", mybir.AluOpType.max, replica_groups=rg,
                ins=[arx_in.opt()], outs=[arx_out.opt()],
            )
            nc.gpsimd.dma_start(axg[:], arx_out[0:1, 0:1])

            trps.release()
            xstage.release()

            col = [8]
            wret = {}
            wretp = [None]
            nld = [0]

            def stage_w_load(bt):
                # alternate the two HWDGE rings (sync/scalar): a single
                # ring's in-order completion handling costs ~0.6us/tile.
                nld[0] += 1
                eng = nc.sync if nld[0] % 2 == 0 else nc.scalar
                ws = wstage.tile([P, 2, N], F32, name="ws", tag="ws")
                eng.dma_start(ws[:], w_pair_src(*bt))
                nc.vector.tensor_reduce(
                    acc[:, col[0]:col[0] + 1], ws[:],
                    axis=mybir.AxisListType.XY, op=mybir.AluOpType.max,
                    apply_absolute_value=True,
                )
                col[0] += 1
                if bt in ret_set:
                    wr = wretp[0].tile([P, 2, N], F16, name="wr", tag="wr")
                    nc.scalar.activation(
                        wr[:], ws[:], mybir.ActivationFunctionType.Copy,
                    )
                    wret[bt] = wr

            for bt in stream_plan[:SX_DEPTH]:
                stage_w_load(bt)

            # sx = 224 / max(amax_x, 1e-12): DVE reaches this ~17 staged
            # reduces deep, by when the AllReduce result has landed.
            nc.vector.tensor_scalar_max(axg[:], axg[:], 1e-12)
            nc.vector.reciprocal(scl[0:1, 0:1], axg[:])
            nc.vector.tensor_scalar_mul(scl[0:1, 1:2], scl[0:1, 0:1], FP8_HALF_MAX)
            nc.gpsimd.partition_broadcast(cb[:, 0:1], scl[0:1, 1:2])
            sx_ap = cb[:, 0:1]

            # xqt quants ride the ACT queue (free once the x drains end);
            # xT's 8MiB then recycles into the w retention pool.
            for b in range(BL):
                nc.scalar.activation(
                    xqt[:, :, :, b * M:(b + 1) * M], xts[b][:],
                    mybir.ActivationFunctionType.Copy, scale=sx_ap,
                )
            xtp.release()
            wretp[0] = tc.alloc_tile_pool(name="wret", bufs=max(retain, 1))

            for bt in stream_plan[SX_DEPTH:]:
                stage_w_load(bt)

            # ---- amax_w AllReduce ----
            nc.vector.tensor_reduce(
                red[:, 1:2], acc[:, 8:col[0]],
                axis=mybir.AxisListType.X, op=mybir.AluOpType.max,
            )
            nc.gpsimd.partition_all_reduce(
                par[:, 1:2], red[:, 1:2], channels=P,
                reduce_op=bass_isa.ReduceOp.max,
            )
            nc.gpsimd.dma_start(arw_in[0:1, 0:1], par[0:1, 1:2])
            nc.gpsimd.collective_compute(
                "AllReduce", mybir.AluOpType.max, replica_groups=rg,
                ins=[arw_in.opt()], outs=[arw_out.opt()],
            )
            nc.gpsimd.dma_start(awg[:], arw_out[0:1, 0:1])
            # sw = 224 / max(amax_w, 1e-12); c = 1/(sx*sw)
            nc.vector.tensor_scalar_max(awg[:], awg[:], 1e-12)
            nc.vector.reciprocal(scl[0:1, 2:3], awg[:])
            nc.vector.tensor_scalar_mul(scl[0:1, 3:4], scl[0:1, 2:3], FP8_HALF_MAX)
            nc.vector.tensor_tensor(
                scl[0:1, 4:5], scl[0:1, 1:2], scl[0:1, 3:4],
                mybir.AluOpType.mult,
            )
            nc.vector.reciprocal(scl[0:1, 5:6], scl[0:1, 4:5])
            nc.gpsimd.partition_broadcast(cb[:, 1:2], scl[0:1, 3:4])
            nc.gpsimd.partition_broadcast(cb[:, 2:3], scl[0:1, 5:6])
            sw_ap = cb[:, 1:2]
            c_ap = cb[:, 2:3]

            # ---- phase B: software-pipelined re-read + quantize + mm ----
            # Engine split keeps every FIFO stall-free: DVE runs ONLY the
            # re-read quants (so the load pipeline is never queued behind
            # drains at batch boundaries); ACT runs the retained-tile
            # quants (fp16, ready as soon as sw lands) plus all drains.
            # Matmuls sweep u-granular (4 PSUM banks), ping-ponged so the
            # PE never waits on a bank drain.
            wqp = tc.alloc_tile_pool(name="wq", bufs=10)
            ostp = tc.alloc_tile_pool(name="ost", bufs=2)
            if with_bias:
                bias1p = tc.alloc_tile_pool(name="bias1", bufs=1)
                biasbp = tc.alloc_tile_pool(name="biasb", bufs=2)

            reread_plan = [bt for bt in flat if bt not in ret_set]
            stage_tiles = {}
            nload = [0]

            def issue_load():
                # prologue rides sync only (the gpsimd queue may be held
                # by the in-flight ARw collective); the steady pipeline
                # alternates sync/gpsimd (the scalar queue is busy with
                # drains, which would head-of-line block triggers).
                if nload[0] >= len(reread_plan):
                    return
                bt = reread_plan[nload[0]]
                eng = nc.sync if (nload[0] < PREFETCH or nload[0] % 2 == 0) \
                    else nc.gpsimd
                st = wstage.tile([P, 2, N], F32, name="ws", tag="ws")
                eng.dma_start(st[:], w_pair_src(*bt))
                stage_tiles[bt] = st
                nload[0] += 1

            for _ in range(PREFETCH):
                issue_load()

            mmps = tc.alloc_tile_pool(name="mmps", bufs=4, space="PSUM")

            wq_all = {}
            for b_, t_ in flat:
                wqt = wqp.tile([P, 2, N], FP8, name="wq", tag="wq")
                if (b_, t_) in ret_set:
                    if with_bias:
                        nc.vector.tensor_scalar(
                            wqt[:], wret[(b_, t_)][:], sw_ap, None,
                            op0=mybir.AluOpType.mult,
                        )
                    else:
                        nc.scalar.activation(
                            wqt[:], wret[(b_, t_)][:],
                            mybir.ActivationFunctionType.Copy, scale=sw_ap,
                        )
                else:
                    nc.vector.tensor_scalar(
                        wqt[:], stage_tiles.pop((b_, t_))[:], sw_ap, None,
                        op0=mybir.AluOpType.mult,
                    )
                    issue_load()
                wq_all[(b_, t_)] = wqt

                if t_ == KP - 1:
                    b = b_
                    if with_bias:
                        b1 = bias1p.tile([1, N], BF16, name="b1", tag="b1")
                        nc.gpsimd.dma_start(b1[:], bias[b, :, :])
                        bb = biasbp.tile([P, N], BF16, name="bb", tag="bb")
                        nc.gpsimd.partition_broadcast(bb[:], b1[:])

                    wq_tiles = [wq_all.pop((b, t)) for t in range(KP)]
                    ost2 = None
                    for u in range(MT):
                        if u % 2 == 0:
                            ost2 = ostp.tile([P, 2, N], BF16,
                                             name="ost", tag="ost")
                        psums = [
                            mmps.tile([P, 2 * NFREE], F32,
                                      name=f"mm{h}", tag="mm")
                            for h in range(NT // 2)
                        ]
                        lo_m = b * M + u * P
                        for t in range(KP):
                            lhsT = xqt[:, t, :, lo_m:lo_m + P]
                            for nt in range(NT):
                                ps = psums[nt // 2]
                                lo = (nt % 2) * NFREE
                                nc.tensor.matmul(
                                    ps[:, lo:lo + NFREE],
                                    lhsT,
                                    wq_tiles[t][:, :,
                                                nt * NFREE:(nt + 1) * NFREE],
                                    start=(t == 0),
                                    stop=(t == KP - 1),
                                    perf_mode=mybir.MatmulPerfMode.DoubleRow,
                                )
                        for h in range(NT // 2):
                            o_ap = ost2[:, u % 2,
                                        h * 2 * NFREE:(h + 1) * 2 * NFREE]
                            if with_bias:
                                nc.vector.scalar_tensor_tensor(
                                    o_ap, psums[h][:], c_ap,
                                    bb[:, h * 2 * NFREE:(h + 1) * 2 * NFREE],
                                    op0=mybir.AluOpType.mult,
                                    op1=mybir.AluOpType.add,
                                )
                            else:
                                nc.scalar.activation(
                                    o_ap, psums[h][:],
                                    mybir.ActivationFunctionType.Copy,
                                    scale=c_ap,
                                )
                        if u % 2 == 1:
                            # plain m-blocks: m = 256*mh + mi*128 + c; the
                            # store rides the scalar HWDGE ring right after
                            # its drains (sync/gpsimd carry the re-reads).
                            mh = u // 2
                            dst = out[b, 2 * mh * P:(2 * mh + 2) * P,
                                      :].rearrange("(r p) n -> p r n", r=2)
                            nc.scalar.dma_start(dst, ost2[:])

            mmps.release()
            if with_bias:
                biasbp.release()
                bias1p.release()
            ostp.release()
            wqp.release()
            wretp[0].release()

    nc.compile()
    return nc


def _get_nc(with_bias):
    key = "fused_b" if with_bias else "fused_nb"
    if key not in _cache:
        _cache[key] = _build_fused_nc(with_bias)
    return _cache[key]


# test.py introspection: exec times (ns) of the last kernel() call.
last_run_info = {}


def kernel(input, weight, bias, _profile=False, _repeat=1, _trace_kwargs=None):
    input = np.ascontiguousarray(input, dtype=np.float32)
    weight = np.ascontiguousarray(weight, dtype=np.float32)
    bias = np.ascontiguousarray(bias, dtype=np.float32)
    assert input.shape == (B, M, K) and weight.shape == (B, K, N)
    assert bias.shape == (B, 1, N)

    consts = np.array([[FP8_HALF_MAX, 1.0]], dtype=np.float32)
    in_maps = [
        {
            "x": input[c * BL:(c + 1) * BL],
            "w": weight[c * BL:(c + 1) * BL],
            "bias": bias[c * BL:(c + 1) * BL],
            "consts": consts,
        }
        for c in range(NCORES)
    ]

    kw = dict(trace=_profile)
    if _trace_kwargs:
        kw.update(_trace_kwargs)

    # bias is exactly zero in this workload; the no-bias NEFF skips the
    # broadcast-add (drains become scaled copies, ACT-assisted at the tail).
    # The with-bias NEFF stays available for correctness on any input.
    nc = _get_nc(with_bias=bool(np.any(bias)))
    times = []
    res = None
    for _ in range(max(1, _repeat)):
        res = run_bass_kernel_spmd(nc, in_maps, core_ids=list(range(NCORES)), **kw)
        times.append(res.exec_time_ns)

    last_run_info.clear()
    last_run_info["amax_times"] = None
    last_run_info["mm_times"] = times
    last_run_info["amax_exec_ns"] = None
    last_run_info["mm_exec_ns"] = min(t for t in times if t) if any(times) else None
    last_run_info["mm_results"] = res

    out = np.concatenate(
        [np.asarray(res.results[c]["out"]).astype(np.float32) for c in range(NCORES)],
        axis=0,
    )
    return out


# revision 26
# speedup vs baseline: 1.0737x; 1.0023x over previous
"""FP8 batch-matmul-dense kernel for Trainium2 (8 NeuronCores, batch-sharded).

Problem: out[b] = fp8qdq(x)[b] @ fp8qdq(w)[b] + bias[b]
  x: [32, 512, 2048] f32, w: [32, 2048, 2048] f32, bias: [32, 1, 2048] f32
  fp8qdq = torchao-style dynamic tensorwise scaling: s = 448/amax(|t|),
  q = e4m3fn(t*s), dq = q/s. Global (whole-tensor) amax.

Sharding: batch axis across 8 cores, 4 slices each (expert-parallel style).

v3 design (single fused NEFF):
  Phase A streams x then w at fp32, computing exact local amaxes on DVE;
  amax_x / amax_w are AllReduce(max)'d (a dummy warmup AllReduce pays the
  first-collective setup under the x loads). x is PE-transposed as it
  arrives and drained to a RAW fp16 xT (8MiB, no scale needed) so the
  transposes never gate on the ARx result; once sx lands, ACT quantizes
  xT -> 4MiB resident fp8 lhsT codes and xT's space is recycled. The tail
  of the w stream (last RETAIN row-pair tiles in stream order) is
  ACT-downcast to resident fp16 (1MiB/tile), cutting the phase-B re-read
  by 2MiB/tile; the stream order is permuted so the retained set spreads
  across batches b1..b3, balancing phase-B DMA per batch against the PE.
  Phase B re-reads only the non-retained w, quantizes on DVE (fp32 for
  re-read tiles, 2x-rate fp16 for retained), and runs DoubleRow fp8
  matmuls (fp32 PSUM accum) in mt-pair sweeps over 8 PSUM banks, drains
  bias+rescale to bf16 and stores via SWDGE (host upcasts).

Performance model (from ntff profiling):
  - The 16 SDMA engines (~22GB/s each on 8-16KB descriptors) bind phase A
    (80MiB: 16 x + 64 w) and roughly tie the PE in phase B (re-read
    50MiB + 8 out vs ~160us of DoubleRow matmul). All tiles move as
    [128, 2, N] row-pairs (one 16KB-contiguous descriptor per partition).
  - Engine queues are strict FIFO: all load triggers ride the sync HWDGE
    ring; the scalar (ACT) queue holds only the x drains / xqt quants /
    retention downcasts, each gated strictly later than the last, so
    nothing head-of-line blocks. sx math sits SX_DEPTH w-reduces deep in
    the DVE FIFO so DVE reaches it just as the ARx result lands.
  - The ARw collective (~40us against a busy SDMA path) is covered by a
    4-deep re-read prefetch prologue into the freed stage slots.

Quantization math (matches the reference lattice exactly): s' = 224/amax
  (= fl(448/amax)/2 exactly) because TRN fp8_e4m3 tops out at 240, not
  448: the OCP e4m3fn lattice scaled by 1/2 lands exactly on the TRN
  lattice. Matmul runs on raw fp8 codes (exact products, fp32 PSUM
  accum); output is rescaled by c = 1/(sx'*sw'). x codes pass through a
  raw fp16 intermediate and retained w tiles are quantized from fp16:
  the extra 2^-11 rounding flips ~0.8% of codes by 1 ulp, adding ~1e-2
  of the 2e-2 relative budget (measured: comfortably inside the gate).

Per-core HBM traffic: 16 (x) + 64 (w) + 50 (w re-read) + 8 (out bf16)
= 138MiB, one NEFF ramp.
"""

import os
import sys

for _p in ("/root/.axon_site", "/root/.axon_site/_ro/trn_rl_repo", "/opt/trn_rl_repo"):
    if os.path.isdir(_p) and _p not in sys.path:
        sys.path.append(_p)

import numpy as np

import concourse.bass as bass
import concourse.bass_isa as bass_isa
import concourse.mybir as mybir
import concourse.tile as tile
from concourse import bacc
from concourse.bass_utils import run_bass_kernel_spmd
from concourse.masks import make_identity

# Problem shape (hardcoded per contest rules).
B, M, K, N = 32, 512, 2048, 2048
NCORES = 8
BL = B // NCORES          # 4 batch slices per core
P = 128
KT = K // P               # 16 k-tiles per batch
KP = KT // 2              # 8 k-groups (256 rows, row-pair packed) per batch
MT = M // P               # 4 m-tiles
NFREE = 512               # matmul moving free dim (one PSUM bank)
NT = N // NFREE           # 4 n-tiles
SX_DEPTH = 18             # staged (2MiB) w reduces before sx in the DVE FIFO
RETAIN = 7                # w k-group tiles retained as fp16 (with_bias: -2)
PREFETCH = 4              # phase-B re-read loads in flight before 1st quant
FP8_HALF_MAX = 224.0      # 448/2: OCP grid mapped onto TRN e4m3

F32 = mybir.dt.float32
F16 = mybir.dt.float16
BF16 = mybir.dt.bfloat16
FP8 = mybir.dt.float8e4

_cache = {}


def _build_fused_nc(with_bias=True):
    nc = bacc.Bacc("TRN2", target_bir_lowering=False, debug=False, num_devices=NCORES)
    x = nc.dram_tensor("x", [BL, M, K], F32, kind="ExternalInput")
    w = nc.dram_tensor("w", [BL, K, N], F32, kind="ExternalInput")
    bias = nc.dram_tensor("bias", [BL, 1, N], F32, kind="ExternalInput")
    consts = nc.dram_tensor("consts", [1, 2], F32, kind="ExternalInput")
    out = nc.dram_tensor("out", [BL, M, N], BF16, kind="ExternalOutput")

    rg = [list(range(NCORES))]
    retain = RETAIN if with_bias is False else RETAIN - 2

    # w stream order: natural order with the retained set moved to the
    # end so retention only needs SBUF after the xT space frees. The
    # retained set spreads over b1..b3 to even phase-B DMA per batch.
    flat = [(b, t) for b in range(BL) for t in range(KP)]
    # spread across batches so every batch keeps ~38us of phase-B re-read
    # DMA to overlap its PE sweeps (b3-heavy retention leaves a pure-
    # compute tail instead)
    ret_set = [(0, 7), (1, 6), (1, 7), (2, 6),
               (2, 7), (3, 6), (3, 7)][-retain:]
    stream_plan = [bt for bt in flat if bt not in ret_set] + ret_set

    def w_pair_src(b, t):
        """w[b] rows [256t, 256t+256) as [128, 2, N]: partition p holds DRAM
        rows 2p/2p+1 -> one 16KB-contiguous descriptor per partition."""
        return w[b, t * 2 * P:(t + 1) * 2 * P, :].rearrange(
            "(p r) n -> p r n", r=2
        )

    def x_half_src(b, s):
        """x[b] rows [256s, 256s+256) as [128, 2, K]: partition p holds
        rows {256s + p, 256s + 128 + p} (plain m-blocks u = 2s, 2s+1)."""
        return x[b, s * 2 * P:(s + 1) * 2 * P, :].rearrange(
            "(u p) n -> p u n", u=2
        )

    with tile.TileContext(nc) as tc:
        with (
            tc.tile_pool(name="small", bufs=1) as small,
            tc.tile_pool(name="acc", bufs=1) as accp,
            tc.tile_pool(name="xqt", bufs=1) as xqtp,
            tc.tile_pool(name="wstage", bufs=4) as wstage,
            tc.tile_pool(name="dram", bufs=6, space="DRAM") as dram,
        ):
            ident = small.tile([P, P], F32, name="ident")
            make_identity(nc, ident[:])
            cst = small.tile([1, 2], F32, name="cst")
            nc.sync.dma_start(cst[:], consts[0:1, :])
            # scl slots: 0=1/ax, 1=sx, 2=1/aw, 3=sw, 4=sx*sw, 5=c
            scl = small.tile([1, 8], F32, name="scl")
            axg = small.tile([1, 1], F32, name="axg")
            awg = small.tile([1, 1], F32, name="awg")
            cb = small.tile([P, 4], F32, name="cb")   # 0=sx, 1=sw, 2=c

            acc = accp.tile([P, 8 + BL * KP], F32, name="acc")
            red = accp.tile([P, 2], F32, name="red")
            par = accp.tile([P, 2], F32, name="par")

            # resident fp8 lhsT codes: [ki, t, par, b*M + u*128 + c] where
            # (ki, par) pair k = 256t + 2*ki + par (matching the w pairing)
            # and unit u = 2s + r pairs m = 256s + 2c + r.
            xqt = xqtp.tile([P, KP, 2, BL * M], FP8, name="xqt")

            dum_in = dram.tile([1, 8], F32, name="dum_in")
            dum_out = dram.tile([1, 8], F32, name="dum_out")
            arx_in = dram.tile([1, 8], F32, name="arx_in")
            arx_out = dram.tile([1, 8], F32, name="arx_out")
            arw_in = dram.tile([1, 8], F32, name="arw_in")
            arw_out = dram.tile([1, 8], F32, name="arw_out")

            # warmup collective: pays the ~80us first-collective setup while
            # the x loads stream.
            nc.gpsimd.dma_start(dum_in[0:1, 0:2], cst[:])
            nc.gpsimd.collective_compute(
                "AllReduce", mybir.AluOpType.max, replica_groups=rg,
                ins=[dum_in.opt()], outs=[dum_out.opt()],
            )

            xtp = tc.alloc_tile_pool(name="xt", bufs=BL)
            xstage = tc.alloc_tile_pool(name="xstage", bufs=2)
            trps = tc.alloc_tile_pool(name="trps", bufs=4, space="PSUM")

            # ---- x: stream, amax, PE-transpose, drain raw fp16 xT ----
            # x rides the sync ring only: the x stream is transpose-paced
            # (~12us/batch), slower than even a single HWDGE ring.
            xts = []
            for b in range(BL):
                views = {}
                for s in range(2):
                    st = xstage.tile([P, 2, K], F32, name="xs", tag="xs")
                    nc.sync.dma_start(st[:], x_half_src(b, s))
                    nc.vector.tensor_reduce(
                        acc[:, 2 * b + s:2 * b + s + 1], st[:],
                        axis=mybir.AxisListType.XY, op=mybir.AluOpType.max,
                        apply_absolute_value=True,
                    )
                    for j in range(2):
                        views[2 * s + j] = st[:, j, :].rearrange(
                            "p (k two) -> p two k", two=2
                        )
                xt = xtp.tile([P, KP, 2, M], F16, name="xt", tag="xt")
                for t in range(KP):
                    # one [P, 1024] psum per k-group: cols parp*512 + u*128
                    # + c, drained fused to xt[:, t, :, :] (fp16, no scale)
                    ps = trps.tile([P, 2 * M], F32, name="tps", tag="tps")
                    for parp in range(2):
                        for u in range(MT):
                            nc.tensor.transpose(
                                ps[:, parp * M + u * P:parp * M + (u + 1) * P],
                                views[u][:, parp, t * P:(t + 1) * P],
                                ident[:],
                            )
                    nc.scalar.activation(
                        xt[:, t, :, :], ps[:],
                        mybir.ActivationFunctionType.Copy,
                    )
                xts.append(xt)

            # ---- amax_x AllReduce trigger (result consumed later) ----
            nc.vector.tensor_reduce(
                red[:, 0:1], acc[:, 0:2 * BL],
                axis=mybir.AxisListType.X, op=mybir.AluOpType.max,
            )
            nc.gpsimd.partition_all_reduce(
                par[:, 0:1], red[:, 0:1], channels=P,
                reduce_op=bass_isa.ReduceOp.max,
            )
            nc.gpsimd.dma_start(arx_in[0:1, 0:1], par[0:1, 0:1])
            nc.gpsimd.collective_compute(
                "AllReduce", mybir.AluOpType.max, replica_groups=rg,
                ins=[arx_in.opt()], outs=[arx_out.opt()],
            )
            nc.gpsimd.dma_start(axg[:], arx_out[0:1, 0:1])

            trps.release()
            xstage.release()

            col = [8]
            wret = {}
            wretp = [None]
            nld = [0]

            def stage_w_load(bt):
                # alternate the two HWDGE rings (sync/scalar): a single
                # ring's in-order completion handling costs ~0.6us/tile.
                nld[0] += 1
                eng = nc.sync if nld[0] % 2 == 0 else nc.scalar
                ws = wstage.tile([P, 2, N], F32, name="ws", tag="ws")
                eng.dma_start(ws[:], w_pair_src(*bt))
                nc.vector.tensor_reduce(
                    acc[:, col[0]:col[0] + 1], ws[:],
                    axis=mybir.AxisListType.XY, op=mybir.AluOpType.max,
                    apply_absolute_value=True,
                )
                col[0] += 1
                if bt in ret_set:
                    wr = wretp[0].tile([P, 2, N], F16, name="wr", tag="wr")
                    nc.scalar.activation(
                        wr[:], ws[:], mybir.ActivationFunctionType.Copy,
                    )
                    wret[bt] = wr

            for bt in stream_plan[:SX_DEPTH]:
                stage_w_load(bt)

            # sx = 224 / max(amax_x, 1e-12): DVE reaches this ~17 staged
            # reduces deep, by when the AllReduce result has landed.
            nc.vector.tensor_scalar_max(axg[:], axg[:], 1e-12)
            nc.vector.reciprocal(scl[0:1, 0:1], axg[:])
            nc.vector.tensor_scalar_mul(scl[0:1, 1:2], scl[0:1, 0:1], FP8_HALF_MAX)
            nc.gpsimd.partition_broadcast(cb[:, 0:1], scl[0:1, 1:2])
            sx_ap = cb[:, 0:1]

            # xqt quants ride DVE right behind the sx math (a ~12us pause
            # the 4-deep stage pool mostly absorbs; the ACT queue would
            # head-of-line block the scalar-ring load triggers instead).
            # xT's 8MiB then recycles into the w retention pool.
            for b in range(BL):
                nc.vector.tensor_scalar(
                    xqt[:, :, :, b * M:(b + 1) * M], xts[b][:], sx_ap, None,
                    op0=mybir.AluOpType.mult,
                )
            xtp.release()
            wretp[0] = tc.alloc_tile_pool(name="wret", bufs=max(retain, 1))

            for bt in stream_plan[SX_DEPTH:]:
                stage_w_load(bt)

            # ---- amax_w AllReduce ----
            nc.vector.tensor_reduce(
                red[:, 1:2], acc[:, 8:col[0]],
                axis=mybir.AxisListType.X, op=mybir.AluOpType.max,
            )
            nc.gpsimd.partition_all_reduce(
                par[:, 1:2], red[:, 1:2], channels=P,
                reduce_op=bass_isa.ReduceOp.max,
            )
            nc.gpsimd.dma_start(arw_in[0:1, 0:1], par[0:1, 1:2])
            nc.gpsimd.collective_compute(
                "AllReduce", mybir.AluOpType.max, replica_groups=rg,
                ins=[arw_in.opt()], outs=[arw_out.opt()],
            )
            nc.gpsimd.dma_start(awg[:], arw_out[0:1, 0:1])
            # sw = 224 / max(amax_w, 1e-12); c = 1/(sx*sw)
            nc.vector.tensor_scalar_max(awg[:], awg[:], 1e-12)
            nc.vector.reciprocal(scl[0:1, 2:3], awg[:])
            nc.vector.tensor_scalar_mul(scl[0:1, 3:4], scl[0:1, 2:3], FP8_HALF_MAX)
            nc.vector.tensor_tensor(
                scl[0:1, 4:5], scl[0:1, 1:2], scl[0:1, 3:4],
                mybir.AluOpType.mult,
            )
            nc.vector.reciprocal(scl[0:1, 5:6], scl[0:1, 4:5])
            nc.gpsimd.partition_broadcast(cb[:, 1:2], scl[0:1, 3:4])
            nc.gpsimd.partition_broadcast(cb[:, 2:3], scl[0:1, 5:6])
            sw_ap = cb[:, 1:2]
            c_ap = cb[:, 2:3]

            # ---- phase B: software-pipelined re-read + quantize + mm ----
            # Engine split keeps every FIFO stall-free: DVE runs ONLY the
            # re-read quants (so the load pipeline is never queued behind
            # drains at batch boundaries); ACT runs the retained-tile
            # quants (fp16, ready as soon as sw lands) plus all drains.
            # Matmuls sweep u-granular (4 PSUM banks), ping-ponged so the
            # PE never waits on a bank drain.
            wqp = tc.alloc_tile_pool(name="wq", bufs=9)
            ostp = tc.alloc_tile_pool(name="ost", bufs=2)
            if with_bias:
                bias1p = tc.alloc_tile_pool(name="bias1", bufs=1)
                biasbp = tc.alloc_tile_pool(name="biasb", bufs=2)

            reread_plan = [bt for bt in flat if bt not in ret_set]
            stage_tiles = {}
            nload = [0]

            def issue_load():
                # prologue rides sync only (the gpsimd queue may be held
                # by the in-flight ARw collective); the steady pipeline
                # alternates sync/gpsimd (the scalar queue is busy with
                # drains, which would head-of-line block triggers).
                if nload[0] >= len(reread_plan):
                    return
                bt = reread_plan[nload[0]]
                eng = nc.sync if (nload[0] < PREFETCH or nload[0] % 2 == 0) \
                    else nc.gpsimd
                st = wstage.tile([P, 2, N], F32, name="ws", tag="ws")
                eng.dma_start(st[:], w_pair_src(*bt))
                stage_tiles[bt] = st
                nload[0] += 1

            for _ in range(PREFETCH):
                issue_load()

            mmps = tc.alloc_tile_pool(name="mmps", bufs=4, space="PSUM")

            wq_all = {}
            for b_, t_ in flat:
                wqt = wqp.tile([P, 2, N], FP8, name="wq", tag="wq")
                if (b_, t_) in ret_set:
                    if with_bias:
                        nc.vector.tensor_scalar(
                            wqt[:], wret[(b_, t_)][:], sw_ap, None,
                            op0=mybir.AluOpType.mult,
                        )
                    else:
                        nc.scalar.activation(
                            wqt[:], wret[(b_, t_)][:],
                            mybir.ActivationFunctionType.Copy, scale=sw_ap,
                        )
                else:
                    nc.vector.tensor_scalar(
                        wqt[:], stage_tiles.pop((b_, t_))[:], sw_ap, None,
                        op0=mybir.AluOpType.mult,
                    )
                    issue_load()
                wq_all[(b_, t_)] = wqt

                if t_ == KP - 1:
                    b = b_
                    if with_bias:
                        b1 = bias1p.tile([1, N], BF16, name="b1", tag="b1")
                        nc.gpsimd.dma_start(b1[:], bias[b, :, :])
                        bb = biasbp.tile([P, N], BF16, name="bb", tag="bb")
                        nc.gpsimd.partition_broadcast(bb[:], b1[:])

                    wq_tiles = [wq_all.pop((b, t)) for t in range(KP)]
                    ost2 = None
                    for u in range(MT):
                        if u % 2 == 0:
                            ost2 = ostp.tile([P, 2, N], BF16,
                                             name="ost", tag="ost")
                        psums = [
                            mmps.tile([P, 2 * NFREE], F32,
                                      name=f"mm{h}", tag="mm")
                            for h in range(NT // 2)
                        ]
                        lo_m = b * M + u * P
                        for t in range(KP):
                            lhsT = xqt[:, t, :, lo_m:lo_m + P]
                            for nt in range(NT):
                                ps = psums[nt // 2]
                                lo = (nt % 2) * NFREE
                                nc.tensor.matmul(
                                    ps[:, lo:lo + NFREE],
                                    lhsT,
                                    wq_tiles[t][:, :,
                                                nt * NFREE:(nt + 1) * NFREE],
                                    start=(t == 0),
                                    stop=(t == KP - 1),
                                    perf_mode=mybir.MatmulPerfMode.DoubleRow,
                                )
                        for h in range(NT // 2):
                            o_ap = ost2[:, u % 2,
                                        h * 2 * NFREE:(h + 1) * 2 * NFREE]
                            if with_bias:
                                nc.vector.scalar_tensor_tensor(
                                    o_ap, psums[h][:], c_ap,
                                    bb[:, h * 2 * NFREE:(h + 1) * 2 * NFREE],
                                    op0=mybir.AluOpType.mult,
                                    op1=mybir.AluOpType.add,
                                )
                            else:
                                nc.scalar.activation(
                                    o_ap, psums[h][:],
                                    mybir.ActivationFunctionType.Copy,
                                    scale=c_ap,
                                )
                        if u % 2 == 1:
                            # plain m-blocks: m = 256*mh + mi*128 + c; the
                            # store rides the scalar HWDGE ring right after
                            # its drains (sync/gpsimd carry the re-reads).
                            mh = u // 2
                            dst = out[b, 2 * mh * P:(2 * mh + 2) * P,
                                      :].rearrange("(r p) n -> p r n", r=2)
                            nc.scalar.dma_start(dst, ost2[:])

            mmps.release()
            if with_bias:
                biasbp.release()
                bias1p.release()
            ostp.release()
            wqp.release()
            wretp[0].release()

    nc.compile()
    return nc


def _get_nc(with_bias):
    key = "fused_b" if with_bias else "fused_nb"
    if key not in _cache:
        _cache[key] = _build_fused_nc(with_bias)
    return _cache[key]


# test.py introspection: exec times (ns) of the last kernel() call.
last_run_info = {}


def kernel(input, weight, bias, _profile=False, _repeat=1, _trace_kwargs=None):
    input = np.ascontiguousarray(input, dtype=np.float32)
    weight = np.ascontiguousarray(weight, dtype=np.float32)
    bias = np.ascontiguousarray(bias, dtype=np.float32)
    assert input.shape == (B, M, K) and weight.shape == (B, K, N)
    assert bias.shape == (B, 1, N)

    consts = np.array([[FP8_HALF_MAX, 1.0]], dtype=np.float32)
    in_maps = [
        {
            "x": input[c * BL:(c + 1) * BL],
            "w": weight[c * BL:(c + 1) * BL],
            "bias": bias[c * BL:(c + 1) * BL],
            "consts": consts,
        }
        for c in range(NCORES)
    ]

    kw = dict(trace=_profile)
    if _trace_kwargs:
        kw.update(_trace_kwargs)

    # bias is exactly zero in this workload; the no-bias NEFF skips the
    # broadcast-add (drains become scaled copies, ACT-assisted at the tail).
    # The with-bias NEFF stays available for correctness on any input.
    nc = _get_nc(with_bias=bool(np.any(bias)))
    times = []
    res = None
    for _ in range(max(1, _repeat)):
        res = run_bass_kernel_spmd(nc, in_maps, core_ids=list(range(NCORES)), **kw)
        times.append(res.exec_time_ns)

    last_run_info.clear()
    last_run_info["amax_times"] = None
    last_run_info["mm_times"] = times
    last_run_info["amax_exec_ns"] = None
    last_run_info["mm_exec_ns"] = min(t for t in times if t) if any(times) else None
    last_run_info["mm_results"] = res

    out = np.concatenate(
        [np.asarray(res.results[c]["out"]).astype(np.float32) for c in range(NCORES)],
        axis=0,
    )
    return out
